# revision 45
# baseline (speedup 1.0000x reference)
"""Bass/Trainium2 kernel for nn_DiagonalTraining (per-anti-diagonal Linear).

Math: for each anti-diagonal i of x[B,S,S] (entries x[b,r,i-r], r<=i),
apply Linear_i (weights W[i,:i+1,:i+1], bias b[i,:i+1]) to the gathered
vector and scatter back reversed. Equivalent to:
    D[b,i,j] = x[b,j,i-j] (j<=i else 0)
    out[b,i,k] = sum_j W[i,k,j] * D[b,i,j] + b[i,k]
    new_x[b,r,c] = out[b,r+c,c] if r+c < S else x[b,r,c]

Device does the einsum (memory-bound: streams the valid triangle of W);
gather/scatter/bias are tiny O(S^2) host ops.

Sharding: interleaved over diagonals — core c owns i = c, c+8, ..., c+504
(slot m holds diagonal 8m+c, k-padded to L=8(m+1)). All cores run one
identical SPMD program; padding rows/cols of W and D are zero by
construction so results are exact.

Performance architecture (v3, measured 32.4-33.2us on HW vs 36.1 baseline;
the measured window includes a fixed ~8.1us walrus semaphore-reset
teardown and ~2.3us of preamble that no kernel code can remove):
- Everything fp8e4: W scaled by 32 on host, D bf16, PSUM result out*32
  staged fp8e4 (total rel err ~1.3e-2 < 2e-2).
- The W image is ONE contiguous [128, WTOT] fp8 buffer laid out in exact
  consumption order. TRIM ("stacked partials"): the mostly-zero last
  j-chunks of paired groups (6,4),(10,8),(14,12),(9,5) share columns at
  partition offsets 96/64 (matmul tile_position=(poff, 32t)), cutting W
  traffic 6.64 -> 6.12MB/core.
- Fetched by ~13 column-range DMAs on the sync+scalar HWDGE queues
  (greedy byte-balanced, issued upfront, each queue leads with a big
  slice then its dt half): sustains 410-420GB/s. Slice sizes taper
  (5400/3000/1600 cols): big early for ramp, small late because the PE
  waits on each slice's completion RECEIPT (~1-2us after last byte).
- Everything SBUF-resident (~59KB/partition); no buffer-reuse waits.
- Matmuls are issued q-outer/t-inner so the four tile_position col-group
  accumulation chains advance in lockstep -- the in-order PE sequencer
  then streams them concurrently (t-outer order serializes the chains:
  21us tensor-busy vs 12.8us interleaved).
- Processing order [15..5, 4, 3..0]: big groups stream; g4's own tiny
  block is the stream tail (fast receipt); groups 3..0 read the image
  HEAD, resident since t0. PSUM: big groups cycle a 6-bank pool, the 5
  tail groups a separate 2-bank pool so the closing chain never waits on
  a late big-group copy (single 8-bank pool put g0 behind g8's copy).
- All PSUM->SBUF copies on vector (scalar ACTIVATE is ~2x slower, and a
  matmul-gated copy scheduled between a W queue's dma_starts stalls the
  fetch issue). Outputs leave in 5 batched DMAs: three mid-stream on
  gpsimd (SWDGE, off the W queues), groups 6..4 on scalar, the final
  320-col batch on sync (both W-idle by then).
"""

import sys

sys.path.insert(0, "/opt/trn_rl_repo")

import numpy as np

B = 8
S = 512
NCORES = 8
M = 64  # diagonal slots per core
LBAR = [8 * (m + 1) for m in range(M)]  # k-padded diagonal length per slot
NQ = [1 if m < 16 else (m // 16 + 1) for m in range(M)]  # j-chunks per slot
QOFF = np.cumsum([0] + NQ).tolist()  # chunk index offset per slot in dt image
DTOT = QOFF[M]  # 160 chunks
G = 16  # groups of 4 slots sharing a PSUM bank
LG = [32 * (g + 1) for g in range(G)]  # group output width
OCUM = np.cumsum([0] + LG).tolist()
OTOT = OCUM[G]  # 4352
WSCALE = 32.0  # fp8 W scale; PSUM holds out*32 which fits fp8e4 directly

# True height of group g's (last) partial j-chunk: 32(g+1) - 128(nq-1).
HTRUE = {g: 32 * (g + 1) - 128 * (NQ[4 * g] - 1) for g in range(G)}

# Processing order: big groups descending (streamed), then groups 3..0
# (W resident at the image HEAD since ~t0 -- they run, cast and flush
# while g4's tail slice is still completing), and finally tiny g4: the
# post-last-byte critical path is its 2 chunk-rounds, one cast and a
# 20KB output.
ORDER = list(range(G - 1, 4, -1)) + [3, 2, 1, 0, 4]

# Stacked-partial trim: (top group, bottom group). Bottom's last chunks sit
# under top's last chunks (same columns, partition offset = top height).
TRIM = True
TRIM_PAIRS = [(6, 4), (10, 8), (14, 12), (9, 5)] if TRIM else []
TOP_OF = {bot: top for top, bot in TRIM_PAIRS}
BOT_OF = {top: bot for top, bot in TRIM_PAIRS}

# Image (stream) order: small groups at the head (resident early, consumed
# late), big groups descending, g4's own block as the tiny stream tail.
IMG_ORDER = [3, 2, 1, 0] + list(range(G - 1, 4, -1)) + [4]

# Output flush batches: (first group, end group, queue) — contiguous OCUM
# ranges keyed by the batch's LAST-processed group. gpsimd takes the
# mid-stream flushes. The five tail groups flush INDIVIDUALLY on
# alternating sync/scalar (W-idle by then): any out DMA's issue->
# completion latency is ~2.6us, so the last one (g4's 20KB) must issue
# as early as possible, not wait for sibling casts in a batch.
OUT_BATCHES = {
    13: (13, 16, "gpsimd"),
    10: (10, 13, "gpsimd"),
    7: (7, 10, "gpsimd"),
    5: (5, 7, "gpsimd"),
    3: (3, 4, "scalar"),
    2: (2, 3, "gpsimd"),
    1: (1, 2, "scalar"),
    0: (0, 1, "gpsimd"),
    4: (4, 5, "sync"),  # sync's ONLY tail out: issues the moment its cast lands
}


def _build_layout():
    """Column layout of the W image in processing order.

    Returns (place, wtot, slices) where place[(m, q)] = (col, poff, h):
    chunk q of slot m lives at image columns [col, col+LBAR[m]) and
    partitions [poff, poff+h). slices = list of (c0, c1) fetch ranges
    aligned to chunk boundaries, sizes tapered big->small.
    """
    place = {}
    col = 0
    bounds = [0]  # chunk-aligned candidate slice boundaries
    breaks = set()  # forced slice breaks (segment edges)

    def put(m, q, poff, h):
        nonlocal col
        place[(m, q)] = (col, poff, h)
        col += LBAR[m]

    # Within a group the image is q-major, t-minor — the matmul issue
    # order — so the PE's four col-group chains advance in lockstep and
    # slices land in consumption order.
    g4_start = [None]
    for g in IMG_ORDER:
        if g == 4:  # the g4 tail block is its own tiny final slice
            breaks.add(col)
            g4_start[0] = col
        nq = NQ[4 * g]
        for q in range(nq - 1):
            for t in range(4):
                m = 4 * g + t
                put(m, q, 0, 128)
                bounds.append(col)
        if g in TOP_OF:
            continue  # bottom group's last chunks live in its top partner
        if g in BOT_OF:
            top_h = HTRUE[g]
            bot = BOT_OF[g]
            bot_h = HTRUE[bot]
            assert top_h % 32 == 0 and top_h + bot_h <= 128
            for t in range(4):
                m = 4 * g + t
                mb = 4 * bot + t
                assert LBAR[mb] <= LBAR[m]
                place[(mb, NQ[mb] - 1)] = (col, top_h, bot_h)
                put(m, NQ[m] - 1, 0, top_h)
                bounds.append(col)
        else:
            for t in range(4):
                m = 4 * g + t
                put(m, NQ[m] - 1, 0, 128)
                bounds.append(col)
    wtot = col

    # (Tried: a gpsimd/SWDGE "opener" slice to start the stream ~1us
    # earlier — regressed ~4us on HW, likely SWDGE descriptor-ring
    # contention during the ramp. Keep the stream on the HWDGE queues.)

    # Split the g4 tail block at its middle chunk boundary: the two mini
    # slices land on opposite queues in parallel, so the final receipt
    # starts earlier and covers half the bytes.
    breaks.add(g4_start[0] + LBAR[16] + LBAR[17])

    # Tapered slice sizes: big early (fast DMA ramp), small late (the PE
    # waits on each slice's completion receipt; cheap small receipts keep
    # the tail tight).
    def target(c):
        f = c / wtot
        return 5400 if f < 0.6 else (3000 if f < 0.85 else 1600)

    slices = []
    prev = 0
    for b_ in bounds:
        if b_ - prev >= target(prev) or (b_ in breaks and b_ > prev):
            slices.append((prev, b_))
            prev = b_
    if prev < wtot:
        slices.append((prev, wtot))
    return place, wtot, slices


PLACE, WTOT, SLICES = _build_layout()

# Greedy byte-balance the fetch slices across the two HWDGE queues, in
# issue order. Each queue also carries half of the dt image upfront.
_DT_COLS = DTOT * B  # bf16 columns


def _assign_queues():
    # dt halves: sync gets [0, dh), scalar [dh, DT_COLS) -- equal bytes.
    dh = _DT_COLS // 2
    load = {"sync": dh * 2, "scalar": (_DT_COLS - dh) * 2}
    out = []
    for c0, c1 in SLICES:
        q = "sync" if load["sync"] <= load["scalar"] else "scalar"
        load[q] += c1 - c0
        out.append((c0, c1, q))
    # Force the last four slices to alternate queues (greedy can stack
    # them on one ring, serializing the tail while the other idles).
    # Ends on scalar so sync is free for g4's final output DMA.
    for i, q in zip(range(-4, 0), ("sync", "scalar", "sync", "scalar")):
        out[i] = (out[i][0], out[i][1], q)
    return dh, out

DT_SPLIT, SLICE_Q = _assign_queues()

MODE = "fp8"
_compiled = {}


def build_program(mode=MODE):
    """Build the SPMD Bass program (same instructions on all 8 cores)."""
    import concourse.mybir as mybir
    import concourse.tile as tile
    from concourse import bacc

    assert mode == "fp8"
    wdt = mybir.dt.float8e4
    ddt = mybir.dt.bfloat16
    odt = mybir.dt.float8e4
    f32 = mybir.dt.float32

    nc = bacc.Bacc("TRN2")
    wimg = nc.dram_tensor("wimg", [128, WTOT], wdt, kind="ExternalInput")
    dt_in = nc.dram_tensor("dt", [128, _DT_COLS], ddt, kind="ExternalInput")
    out = nc.dram_tensor("out", [128, OTOT], odt, kind="ExternalOutput")

    with tile.TileContext(nc) as tc:
        with (
            tc.tile_pool(name="dpool", bufs=1) as dpool,
            tc.tile_pool(name="wpool", bufs=1) as wpool,
            tc.tile_pool(name="opool", bufs=1) as opool,
            tc.tile_pool(name="psum", bufs=4, space="PSUM") as psum_pool,
            tc.tile_pool(name="psum_sm", bufs=3, space="PSUM") as psum_sm,
            tc.tile_pool(name="psum_g4", bufs=1, space="PSUM") as psum_g4,
        ):
            dtall = dpool.tile([128, _DT_COLS], ddt)
            wtile = wpool.tile([128, WTOT], wdt)
            ostage = opool.tile([128, OTOT], odt)

            # Each queue leads with a big W slice (best DMA ramp), then its
            # dt half, then the rest of its slices in processing order.
            ndma = {"sync": 0, "scalar": 0, "gpsimd": 0}
            engs = {"sync": nc.sync, "scalar": nc.scalar, "gpsimd": nc.gpsimd}
            for c0, c1, qname in SLICE_Q:
                eng = engs[qname]
                eng.dma_start(wtile[:, c0:c1], wimg[:, c0:c1])
                ndma[qname] += 1
                if ndma["sync"] == 1 and qname == "sync":
                    nc.sync.dma_start(dtall[:, 0:DT_SPLIT], dt_in[:, 0:DT_SPLIT])
                if ndma["scalar"] == 1 and qname == "scalar":
                    nc.scalar.dma_start(dtall[:, DT_SPLIT:], dt_in[:, DT_SPLIT:])

            for g in ORDER:
                # PSUM banks: big groups cycle 4 (reuse distance ~6-9us of
                # stream time -- safe); tail groups 3..0 cycle their own 3
                # so they never wait on a late big-group copy; g4 -- the
                # last-processed group -- has a dedicated bank so its
                # matmuls wait on nothing but its own W slice.
                pool = psum_g4 if g == 4 else (psum_sm if g <= 3 else psum_pool)
                ps = pool.tile([128, 512], f32, tag="ps")
                nq = NQ[4 * g]
                # q-outer / t-inner: the four col-group accumulation chains
                # advance in lockstep so the PE streams them concurrently.
                for q in range(nq):
                    for t in range(4):
                        m = 4 * g + t
                        L = LBAR[m]
                        c0, poff, h = PLACE[(m, q)]
                        nc.tensor.matmul(
                            ps[32 * t : 32 * t + B, 0:L],
                            lhsT=dtall[
                                poff : poff + h,
                                (QOFF[m] + q) * B : (QOFF[m] + q + 1) * B,
                            ],
                            rhs=wtile[poff : poff + h, c0 : c0 + L],
                            start=(q == 0),
                            stop=(q == nq - 1),
                            tile_position=(poff, 32 * t),
                        )
                oslice = ostage[0:128, OCUM[g] : OCUM[g] + LG[g]]
                # ALL copies on vector: a copy on scalar/sync could be
                # scheduled between that engine's W dma_starts and its
                # matmul-gated wait would then stall W fetch issue
                # (in-order sequencer). Vector does nothing else.
                nc.vector.tensor_copy(oslice, ps[0:128, 0 : LG[g]])
                if g in OUT_BATCHES:
                    glo, ghi, qname = OUT_BATCHES[g]
                    eng = {"sync": nc.sync, "scalar": nc.scalar}.get(
                        qname, nc.gpsimd
                    )
                    eng.dma_start(
                        out[:, OCUM[glo] : OCUM[ghi]],
                        ostage[0:128, OCUM[glo] : OCUM[ghi]],
                    )

    nc.compile()
    return nc


def _get_program(mode=MODE):
    if mode not in _compiled:
        _compiled[mode] = build_program(mode)
    return _compiled[mode]


def _prep_inputs(x, W, mode=MODE):
    """Host-side shard prep: gather diagonals of x, pack W/D images."""
    import ml_dtypes

    wnp = np.dtype(ml_dtypes.float8_e4m3)
    dnp = np.dtype(ml_dtypes.bfloat16)
    wscale = np.float32(WSCALE)

    i_idx = np.arange(S)[:, None]
    r_idx = np.arange(S)[None, :]
    cols = (i_idx - r_idx) % S
    valid = (r_idx <= i_idx)[None]
    D = np.where(valid, x[:, r_idx, cols], np.float32(0.0))  # [B, S(i), S(j)]

    in_maps = []
    for c in range(NCORES):
        Wc = W[c::8]  # [M, S(k), S(j)]
        WIMG = np.zeros((128, WTOT), dtype=wnp)
        for m in range(M):
            L, nq = LBAR[m], NQ[m]
            for q in range(nq):
                c0, poff, h = PLACE[(m, q)]
                # img[j, k] = Wc[m, k, 128q + j] * wscale
                blk = Wc[m, 0:L, 128 * q : 128 * (q + 1)] * wscale  # [k, j]
                img = blk.T.astype(wnp, copy=False)  # [j<=128, k=L]
                jh = min(img.shape[0], h)
                WIMG[poff : poff + jh, c0 : c0 + L] = img[0:jh]
        # DT[j, (QOFF[m]+q)*B + b] = D[b, 8m+c, 128q+j]; chunks mirror the
        # W image's partition offsets.
        Dc = D[:, c::8, :]  # [B, M, S]
        DT = np.zeros((128, _DT_COLS), dtype=dnp)
        for m in range(M):
            nq = NQ[m]
            for q in range(nq):
                _, poff, h = PLACE[(m, q)]
                arr = Dc[:, m, 128 * q : 128 * (q + 1)].T  # [j<=128, B]
                jh = min(arr.shape[0], h)
                DT[poff : poff + jh, (QOFF[m] + q) * B : (QOFF[m] + q + 1) * B] = (
                    arr[0:jh].astype(dnp, copy=False)
                )
        in_maps.append({"wimg": WIMG, "dt": DT})
    return in_maps


def _postprocess(x, bvec, results, mode=MODE):
    """Assemble per-core outputs, undo W scale, add bias, scatter back."""
    inv_scale = np.float32(1.0 / WSCALE)
    out_full = np.empty((B, S, S), dtype=np.float32)
    for c in range(NCORES):
        o = np.asarray(results[c]["out"]).astype(np.float32)  # [128, OTOT]
        for g in range(G):
            blk = o[:, OCUM[g] : OCUM[g + 1]].reshape(4, 32, LG[g])[:, 0:B]
            for t in range(4):
                m = 4 * g + t
                out_full[:, 8 * m + c, 0 : LBAR[m]] = blk[t, :, 0 : LBAR[m]]
    out_full *= inv_scale
    out_full += bvec[None]
    rr = np.arange(S)[:, None]
    cc = np.arange(S)[None, :]
    diag = rr + cc
    new_x = np.where(
        (diag < S)[None], out_full[:, np.minimum(diag, S - 1), cc], x
    ).astype(np.float32)
    return new_x


def kernel_run(x, W, b, mode=MODE, trace=False):
    from concourse.bass_utils import run_bass_kernel_spmd

    nc = _get_program(mode)
    in_maps = _prep_inputs(x, W, mode)
    res = run_bass_kernel_spmd(nc, in_maps, list(range(NCORES)), trace=trace)
    return _postprocess(x, b, res.results, mode), res


def kernel(x, W, b):
    out, _ = kernel_run(np.asarray(x), np.asarray(W), np.asarray(b))
    return out
